# revision 36
# baseline (speedup 1.0000x reference)
"""Multi-head attention (b=4, c=256, l=2048, 8 heads x 64) on 8 TRN2 NeuronCores.

Sharding: core i handles batch b = i//2 and query half qh = i%2 (1024 queries),
computing all 8 heads over the full 2048-key context. Outputs are disjoint
[256, 1024] slabs -> host-side concat only, no collectives. The core's query
half is ROLLED to the front of its x copy host-side (keys are permuted
consistently for K and V, softmax is permutation-invariant), so no separate
xq input is sent.

Hard-won HW facts this kernel is built around (measured via microbenches):
 1. CLOCK POISON: the mere presence of InstReciprocal, gpsimd
    partition_broadcast, or scalar.activation(Identity, bias=AP) anywhere in
    the program drops EVERY engine's clock 1.2x (exp 1540->1848ns, matmul
    377->451ns). The softmax normalization is rebuilt poison-free on DVE:
    reciprocal_approx_fast (input MUST be at partition offset 0 -- the
    custom-DVE op misreads offset inputs, hence the den staging copy) +
    STREAM_SHUFFLE broadcast (mask=0s within a 32-block) + offset copy + TT.
 2. ACT cost = (N + ~310)/1.2GHz per ACTIVATE, linear in width, dtype- and
    accum-independent. 16.78M exps/core / 128 lanes => ~131us floor; it runs
    ONLY exp (scale=1/sqrt(d) folded into the ACTIVATE immediate).
 3. PSUM bank-PAIR port conflicts: an ACT read and a concurrent PE write to
    the same bank pair cost ~+12%. Segs 2-7 alternate 2-bank [128,1024] and
    4-bank [128,2048] score tiles (2+4+2 pv = 8 banks, pair-aligned) so the
    read tile and write tile never share a pair. Segs 0-1 use 2x 1024-wide
    tiles + 2 proj banks: the pair 1-3 projections run as per-chunk tasks
    INSIDE the stream (PV flush lags 4 chunks in seg0 while VT tiles are
    projected just-in-time, then drains to lag 1).
 4. DMA engines take ~5us from first descriptor to first packet, then move
    ~2MB in ~8us; critical slices (pair-0 weights, first x columns) are
    issued first, interleaved across the sync/gpsimd queues. Output
    transfers are spread over sync/gpsimd/scalar queues to parallelize the
    final drain. fp8 was evaluated and rejected: V/E/Q/K quantization noise
    propagates ~linearly to the output (does not average down), far above
    the 2e-2 budget. PE is pre-warmed with dummy matmuls during the DMA
    lead-in so the first QK chunks run near full clock.

Measured: 170.5-171.5us vs 174.5-174.6us for the session-start checkpoint
(pair-timed in-process, fast device state; ~203-205us in the 1.18x-slow
device state). Since the last checkpoint: 12 wider PE warmups keep the
p-state at full clock through the DMA lead-in (-1.0%), output DMAs moved
off the gpsimd SWDGE queue (its ~3.5us teardown DRAIN) onto sync/scalar
HWDGE queues, and TWO mid-seg 1024-wide chunks per phase-2 seg (ci 5 and 8 -- away from
the boundary norm burst and the qk ladder corners) run the Schraudolph exp
on DVE (phi=9.4%, rel err 5.054e-3 -> 5.515e-3), relieving the ACT cadence
(-0.7% combined). 16 warmups instead of 12 measured WORSE (+0.5%: they
delay the first projection); 12 is the sweet spot. Earlier this revision: ~7us preamble + ~10us DMA/proj
ramp, ~137.5us exp stream, ~18.5us tail. Tail improvements this revision:
the last segment's den-staging and rbc-expand copies run on ACT (idle after
its final exp; activation-Copy with a FLOAT bias carries no clock poison --
only Identity with bias=AP does), overlapping the serial DVE norm chain;
and the output bias-add/convert is split per column half so each of the
four output DMAs launches as soon as its half exists. The two
overflow projection tasks are double-popped under seg1's final exp chunks
rather than bursting at the seg1->seg2 PSUM-pool transition, and the PV
flush lag stays at 3 through both task-laden segments (0-1), draining to 1
only in seg2 where the PE carries no projection work.

Optimization routes EXPLORED AND REJECTED (2026-08-10 session; all measured
on HW against this baseline pair-timed in the same process):
 - The stream is already ~PE-paced: segs 2-7 complete every ~15.35us vs
   ~14.5us of PE matmul issue time; ACT's 140us busy mostly hides inside it.
   PE work (QK 54.6 + PV 54.6 + proj 17.1 + outproj 4.3us) is irreducible at
   bf16 (PV's 65-row output wastes partitions, not cycles; fp8 anywhere
   breaks the 2e-2 budget).
 - Schraudolph exp on DVE (bf16_bits(i16(23.083*s + 16250.5)) == exp(s/8),
   one tensor_scalar, 2.06% rms sawtooth/elem, only ~8.9e-3*sqrt(phi)
   end-to-end after softmax cancellation) WORKS numerically and adds no
   clock poison, but offloading ACT does not shrink a PE-paced stream; the
   2-deep qk-tile rotation couples each chunk's exp completion to QK(c+2),
   so DVE-queue jitter (norm bursts) turns into PE stalls + p-state drops.
   Measured 177.6-183.9us on several structures vs 174-175 baseline.
 - Query-half-major seg order ((0,0),(1,0),...) + per-p out-proj PSUM
   accumulation + mid-stream output DMA: no banks left for a persistent
   out-proj pool (pv 2 + qk 6 in phase 2), and the Tile scheduler hoists
   end-emitted out-proj into the stream on its own anyway.
 - GPSIMD cannot access PSUM (BIR verifier). SBUF-only gpsimd TTs work
   (0.42 eff) and do not poison the clock; useful only off the tail path.
 - Producer->consumer margins below ~2 chunks between a DVE cast and its
   PE reader intermittently RACE on HW (partially-written reads, ~20% of
   fresh processes; scheduler sim does not catch it). Keep >=2.5 chunks.
 - Device clock varies between sessions: the same NEFF measures 174-175us
   (fast state) or ~206-220us (1.18x slow state, matmuls 450ns vs 386ns).
   Always pair-time any candidate against this baseline in-process.
"""

import sys

if "/opt/trn_rl_repo" not in sys.path:
    sys.path.insert(0, "/opt/trn_rl_repo")

import numpy as np

import concourse.bass as bass
import concourse.mybir as mybir
import concourse.tile as tile
from concourse import bacc
from concourse.bass_utils import run_bass_kernel_spmd

F32 = mybir.dt.float32
BF16 = mybir.dt.bfloat16
I16 = mybir.dt.int16
EXP = mybir.ActivationFunctionType.Exp
IDENT = mybir.ActivationFunctionType.Identity
MULT = mybir.AluOpType.mult
ADD = mybir.AluOpType.add

B, C, L = 4, 256, 2048
H, D = 8, 64
HID = H * D  # 512
LQ = L // 2  # 1024 queries per core
NJT = L // 128  # 16 key tiles
SCALE = D**-0.5
STREAM_TASKS = True
# Schraudolph exp via the bf16 exponent field: bf16_bits(i16(A*s+B)) ~=
# exp(SCALE*s); 2.06% rms/elem but only ~8.9e-3*sqrt(phi) end-to-end after
# softmax cancellation. Used on ONE mid-seg 1024-wide chunk per phase-2 seg.
SCH_A = SCALE * 1.4426950408889634 * 128.0
SCH_B = 16256.0 - 5.513

_cached = {}


def build_nc():
    nc = bacc.Bacc(
        "TRN2",
        target_bir_lowering=False,
        debug=False,
        enable_asserts=False,
        num_devices=8,
    )
    x_d = nc.dram_tensor("x", [C, L], BF16, kind="ExternalInput")
    wq_d = nc.dram_tensor("wqkvT", [C, 3 * HID], BF16, kind="ExternalInput")
    wo_d = nc.dram_tensor("woutT", [HID, C], BF16, kind="ExternalInput")
    bias_d = nc.dram_tensor("bias", [C, 1], F32, kind="ExternalInput")
    out_d = nc.dram_tensor("out", [C, LQ], BF16, kind="ExternalOutput")

    with tile.TileContext(nc) as tc:
        with (
            tc.tile_pool(name="const", bufs=1) as cp,
            tc.tile_pool(name="epool", bufs=8) as ep,
            tc.tile_pool(name="opool", bufs=2) as op,
            tc.tile_pool(name="pvps", bufs=1, space=bass.MemorySpace.PSUM) as pvps,
        ):
            # ---- persistent SBUF tensors ----
            xb = [cp.tile([128, L], BF16, tag=f"xb{k}", name=f"xb{k}") for k in range(2)]
            wq = [cp.tile([128, 3 * HID], BF16, tag=f"wq{k}", name=f"wq{k}") for k in range(2)]
            wo = [cp.tile([128, C], BF16, tag=f"wo{k}", name=f"wo{k}") for k in range(4)]
            bias = [cp.tile([128, 1], F32, tag=f"bias{k}", name=f"bias{k}") for k in range(2)]
            Qs = [cp.tile([128, LQ], BF16, tag=f"Q{m}", name=f"Q{m}") for m in range(4)]
            Ks = [cp.tile([128, L], BF16, tag=f"K{m}", name=f"K{m}") for m in range(4)]
            VT = [cp.tile([128, H, D + 1], BF16, tag=f"VT{t}", name=f"VT{t}") for t in range(NJT)]
            attn = [cp.tile([128, LQ], BF16, tag=f"attn{m}", name=f"attn{m}") for m in range(4)]
            pons = [cp.tile([D + 1, 512], F32, tag=f"pons{k}", name=f"pons{k}") for k in range(4)]
            dens = [cp.tile([1, 512], F32, tag=f"den{k}", name=f"den{k}") for k in range(4)]
            recs = [cp.tile([32, 512], F32, tag=f"rec{k}", name=f"rec{k}") for k in range(2)]
            rbcs = [cp.tile([64, 512], F32, tag=f"rbc{k}", name=f"rbc{k}") for k in range(4)]
            dum = cp.tile([1, 16], F32, tag="dum", name="dum")
            dumo = cp.tile([1, 16], F32, tag="dumo", name="dumo")
            wdum = cp.tile([128, 128], BF16, tag="wdum", name="wdum")
            rdum = cp.tile([128, 512], BF16, tag="rdum", name="rdum")

            # warmups: exp table load on ACT; PE clock ramp dummies come after
            # the pools open (they need psum) -- see below.
            nc.vector.memset(dum[:], 1.0)
            nc.vector.memset(wdum[:], 0.125)
            nc.vector.memset(rdum[:], 0.125)
            nc.scalar.activation(dumo[:], dum[:], EXP)
            for k in range(2):
                nc.vector.memset(recs[k][:], 0.0)

            # ---- DMA: critical slices interleaved across sync+gpsimd queues
            nc.sync.dma_start(wq[0][:, 0:128], wq_d.ap()[0:128, 0:128])
            nc.gpsimd.dma_start(wq[1][:, 0:128], wq_d.ap()[128:256, 0:128])
            nc.sync.dma_start(wq[0][:, 512:640], wq_d.ap()[0:128, 512:640])
            nc.gpsimd.dma_start(wq[1][:, 512:640], wq_d.ap()[128:256, 512:640])
            nc.sync.dma_start(xb[0][:, 0:512], x_d.ap()[0:128, 0:512])
            nc.gpsimd.dma_start(xb[1][:, 0:512], x_d.ap()[128:256, 0:512])
            nc.sync.dma_start(wq[0][:, 1024:1536], wq_d.ap()[0:128, 1024:1536])
            nc.gpsimd.dma_start(wq[1][:, 1024:1536], wq_d.ap()[128:256, 1024:1536])
            nc.sync.dma_start(xb[0][:, 512:1024], x_d.ap()[0:128, 512:1024])
            nc.gpsimd.dma_start(xb[1][:, 512:1024], x_d.ap()[128:256, 512:1024])
            # VT ones columns (no deps)
            for t in range(NJT):
                nc.gpsimd.memset(VT[t][:, :, D : D + 1], 1.0)
            # bulk
            nc.sync.dma_start(xb[0][:, 1024:2048], x_d.ap()[0:128, 1024:2048])
            nc.gpsimd.dma_start(xb[1][:, 1024:2048], x_d.ap()[128:256, 1024:2048])
            nc.sync.dma_start(wq[0][:, 128:512], wq_d.ap()[0:128, 128:512])
            nc.gpsimd.dma_start(wq[1][:, 128:512], wq_d.ap()[128:256, 128:512])
            nc.sync.dma_start(wq[0][:, 640:1024], wq_d.ap()[0:128, 640:1024])
            nc.gpsimd.dma_start(wq[1][:, 640:1024], wq_d.ap()[128:256, 640:1024])
            for k in range(4):
                q = nc.sync if k % 2 == 0 else nc.gpsimd
                q.dma_start(wo[k][:], wo_d.ap()[128 * k : 128 * (k + 1), :])
            for k in range(2):
                rows = slice(128 * k, 128 * (k + 1))
                q = nc.sync if k % 2 == 0 else nc.gpsimd
                q.dma_start(bias[k][:], bias_d.ap()[rows, :])

            # ---- projection task machinery ----
            task_pool = [None]  # set once proj psum pool opens

            def q_task(p, n):
                ps = task_pool[0].tile([128, 512], F32, tag="proj", name="psq")
                for k in range(2):
                    nc.tensor.matmul(
                        ps[:],
                        wq[k][:, 128 * p : 128 * (p + 1)],
                        xb[k][:, 512 * n : 512 * (n + 1)],
                        start=(k == 0),
                        stop=(k == 1),
                    )
                nc.vector.tensor_copy(Qs[p][:, 512 * n : 512 * (n + 1)], ps[:])

            def k_task(p, j):
                ps = task_pool[0].tile([128, 512], F32, tag="proj", name="psk")
                for k in range(2):
                    nc.tensor.matmul(
                        ps[:],
                        wq[k][:, HID + 128 * p : HID + 128 * (p + 1)],
                        xb[k][:, 512 * j : 512 * (j + 1)],
                        start=(k == 0),
                        stop=(k == 1),
                    )
                nc.vector.tensor_copy(Ks[p][:, 512 * j : 512 * (j + 1)], ps[:])

            def vt_task(t):
                ps = task_pool[0].tile([128, 512], F32, tag="proj", name="psv")
                for k in range(2):
                    nc.tensor.matmul(
                        ps[:],
                        xb[k][:, 128 * t : 128 * (t + 1)],
                        wq[k][:, 2 * HID : 3 * HID],
                        start=(k == 0),
                        stop=(k == 1),
                    )
                nc.vector.tensor_copy(
                    VT[t][:, :, 0:D], ps[:].rearrange("p (h c) -> p h c", h=H)
                )

            tasks = (
                [lambda: k_task(0, 1), lambda: vt_task(4), lambda: vt_task(5),
                 lambda: k_task(0, 2), lambda: vt_task(6), lambda: vt_task(7),
                 lambda: vt_task(8), lambda: k_task(0, 3), lambda: q_task(0, 1),
                 lambda: vt_task(9), lambda: vt_task(10), lambda: vt_task(11),
                 lambda: vt_task(12), lambda: vt_task(13), lambda: vt_task(14),
                 lambda: vt_task(15)]
                + [lambda: k_task(1, 0), lambda: k_task(1, 1), lambda: k_task(1, 2),
                   lambda: k_task(1, 3), lambda: q_task(1, 0), lambda: q_task(1, 1),
                   lambda: k_task(2, 0), lambda: k_task(2, 1), lambda: k_task(2, 2),
                   lambda: k_task(2, 3), lambda: q_task(2, 0), lambda: q_task(2, 1),
                   lambda: k_task(3, 0), lambda: k_task(3, 1), lambda: k_task(3, 2),
                   lambda: k_task(3, 3)]
                + [lambda: q_task(3, 0), lambda: q_task(3, 1)]
            )
            task_i = [0]

            def pop_task():
                if task_i[0] < len(tasks):
                    tasks[task_i[0]]()
                    task_i[0] += 1

            # ---- normalization chain (poison-free: no InstReciprocal, no
            # partition_broadcast) ----
            norm_i = [0]

            def do_norm(src_pair, p, ih):
                cols = slice(512 * ih, 512 * (ih + 1))
                for s in (0, 1):
                    i = norm_i[0]
                    norm_i[0] += 1
                    rec = recs[i % 2]
                    rbc = rbcs[i % 4]
                    # custom-DVE ops misread partition-offset inputs: stage the
                    # denominator row at partition 0 before the approx recip.
                    den = dens[i % 4]
                    nc.vector.tensor_copy(den[:], src_pair[s][D : D + 1, :])
                    nc.vector.reciprocal_approx_fast(rec[0:1, :], den[:])
                    nc.vector.stream_shuffle(rbc[0:32, :], rec[0:32, :], [0] * 32)
                    nc.vector.tensor_copy(rbc[32:64, :], rbc[0:32, :])
                    nc.vector.tensor_tensor(
                        attn[p][64 * s : 64 * (s + 1), cols],
                        src_pair[s][0:D, :],
                        rbc[:],
                        MULT,
                    )

            def pv_flush(E, u0, nu, po, p):
                for ui in range(nu):
                    jt, s = divmod(u0 + ui, 2)
                    nc.tensor.matmul(
                        po[s][:],
                        VT[jt][:, 2 * p + s, :],
                        E[:, 512 * ui : 512 * (ui + 1)],
                        start=(jt == 0),
                        stop=(jt == NJT - 1),
                    )

            def seg_finish(po, p, ih, seg):
                if seg == 7:
                    # nothing follows: normalize straight from PV psum. The
                    # den staging + rbc expand copies run on ACT (idle after
                    # its last exp; activation-Copy with float bias carries no
                    # clock poison), halving the serial DVE chain.
                    cols = slice(512 * ih, 512 * (ih + 1))
                    for s in (0, 1):
                        i = norm_i[0]
                        norm_i[0] += 1
                        rec = recs[i % 2]
                        rbc = rbcs[i % 4]
                        den = dens[i % 4]
                        nc.scalar.copy(den[:], po[s][D : D + 1, :])
                        nc.vector.reciprocal_approx_fast(rec[0:1, :], den[:])
                        nc.vector.stream_shuffle(rbc[0:32, :], rec[0:32, :], [0] * 32)
                        nc.scalar.copy(rbc[32:64, :], rbc[0:32, :])
                        nc.vector.tensor_tensor(
                            attn[p][64 * s : 64 * (s + 1), cols],
                            po[s][0:D, :],
                            rbc[:],
                            MULT,
                        )
                    return
                pp = [pons[2 * (seg % 2) + s] for s in (0, 1)]
                for s in (0, 1):
                    nc.vector.tensor_copy(pp[s][:], po[s][:])
                do_norm(pp, p, ih)

            # ---- the stream ----
            pending = []  # FIFO of (E, u0, nu, po, p, ih, seg)

            def run_seg(seg, chunk_plan):
                p, ih = divmod(seg, 2)
                Qh = [
                    Qs[p][64 * s : 64 * (s + 1), 512 * ih : 512 * (ih + 1)]
                    for s in (0, 1)
                ]
                Kh = [Ks[p][64 * s : 64 * (s + 1), :] for s in (0, 1)]
                po = [
                    pvps.tile([D + 1, 512], F32, tag=f"pv{s}", name=f"po{s}")
                    for s in (0, 1)
                ]
                # keep the PV flush lag deep through the task-laden segs 0-1;
                # drain to lag 1 in seg2 where the PE has no projection work
                lag = 3 if seg <= 1 else 1
                u0 = 0
                for ci, (qkps, tile_w, nuc) in enumerate(chunk_plan):
                    ps = qkps.tile([128, tile_w], F32, tag="qk", name="psqk")
                    for ui in range(nuc):
                        jt, s = divmod(u0 + ui, 2)
                        nc.tensor.matmul(
                            ps[:, 512 * ui : 512 * (ui + 1)],
                            Kh[s][:, 128 * jt : 128 * (jt + 1)],
                            Qh[s][:],
                            start=True,
                            stop=True,
                        )
                    pops = 0
                    while pending and len(pending) > lag and pops < 2:
                        ent = pending.pop(0)
                        pv_flush(*ent[:5])
                        if ent[1] + ent[2] == 2 * NJT:
                            seg_finish(ent[3], ent[4], ent[5], ent[6])
                        pops += 1
                    E = ep.tile([128, 2048], BF16, tag="e", name="E")
                    if seg >= 2 and tile_w == 1024 and ci in (5, 8):
                        # one mid-seg chunk on DVE relieves the ACT cadence
                        # without exposing the qk-tile ladder to DVE jitter
                        nc.vector.tensor_scalar(
                            E[:, 0 : 512 * nuc].bitcast(I16),
                            ps[:, 0 : 512 * nuc], SCH_A, SCH_B, MULT, ADD)
                    else:
                        nc.scalar.activation(E[:, 0 : 512 * nuc],
                                             ps[:, 0 : 512 * nuc], EXP, scale=SCALE)
                    pending.append((E, u0, nuc, po, p, ih, seg))
                    if STREAM_TASKS and seg < 2:
                        pop_task()
                        # absorb the 2 overflow tasks under seg1's final exps
                        # instead of bursting them at the phase transition
                        if seg == 1 and ci >= 14:
                            pop_task()
                    u0 += nuc
                assert u0 == 2 * NJT

            with (
                tc.tile_pool(name="projps", bufs=2, space=bass.MemorySpace.PSUM) as projp,
                tc.tile_pool(name="qkA", bufs=2, space=bass.MemorySpace.PSUM) as qkA,
            ):
                task_pool[0] = projp
                # PE clock-ramp warmup during the DMA lead-in (no input deps)
                # 12 wider warmups: keep PE busy through the whole DMA
                # lead-in so its p-state is at full clock when the first
                # real projection matmul lands.
                for i in range(12):
                    ps = qkA.tile([128, 1024], F32, tag="qk", name="warm")
                    nc.tensor.matmul(ps[:, 0:512], wdum[:], rdum[:],
                                     start=True, stop=True)
                # minimal phase-1: pair0 head tiles + first VTs
                q_task(0, 0)
                k_task(0, 0)
                vt_task(0)
                vt_task(1)
                vt_task(2)
                vt_task(3)
                if not STREAM_TASKS:
                    while task_i[0] < len(tasks):
                        pop_task()
                for seg in range(2):
                    run_seg(seg, [(qkA, 1024, 2)] * 16)
                # leftover projection tasks (their DMA landed long ago)
                while task_i[0] < len(tasks):
                    pop_task()
            with (
                tc.tile_pool(name="qkB2", bufs=1, space=bass.MemorySpace.PSUM) as qkB2,
                tc.tile_pool(name="qkB1", bufs=1, space=bass.MemorySpace.PSUM) as qkB1,
            ):
                # alternate 2-bank and 4-bank score tiles: the ACT-read tile
                # and the PE-write tile never share a PSUM bank pair. Per-seg
                # patterns keep the alternation strict across seg boundaries.
                # qkB2 is declared first so it lands on the banks the qkA pool
                # vacates earliest at the phase transition.
                planA = [(qkB1, 1024, 2), (qkB2, 2048, 4)] * 5 + [(qkB1, 1024, 2)]
                planB = [(qkB2, 2048, 4), (qkB1, 1024, 2)] * 5 + [(qkB2, 2048, 2)]
                for seg in range(2, 8):
                    run_seg(seg, planB if seg % 2 == 0 else planA)
                for ent in pending:
                    pv_flush(*ent[:5])
                    if ent[1] + ent[2] == 2 * NJT:
                        seg_finish(ent[3], ent[4], ent[5], ent[6])

            # ---- output projection ----
            with tc.tile_pool(name="ops", bufs=2, space=bass.MemorySpace.PSUM) as ops:
                for m in range(2):
                    ps = ops.tile([128, LQ], F32, tag="o", name="pso")
                    for k in range(3):
                        for n in range(2):
                            nc.tensor.matmul(
                                ps[:, 512 * n : 512 * (n + 1)],
                                wo[k][:, 128 * m : 128 * (m + 1)],
                                attn[k][:, 512 * n : 512 * (n + 1)],
                                start=(k == 0),
                                stop=False,
                            )
                    for half in range(2):
                        hr = slice(64 * half, 64 * (half + 1))
                        for n in range(2):
                            nc.tensor.matmul(
                                ps[:, 512 * n : 512 * (n + 1)],
                                wo[3][hr, 128 * m : 128 * (m + 1)],
                                attn[3][hr, 512 * n : 512 * (n + 1)],
                                start=False,
                                stop=(half == 1),
                            )
                    osb = op.tile([128, LQ], BF16, tag="osb", name="osb")
                    # NOTE: scalar.activation(Identity, bias=AP) triggers the
                    # same global 1.2x clock slowdown as InstReciprocal and
                    # partition_broadcast -- keep bias adds on DVE.
                    # Convert per column half and launch each transfer as soon
                    # as its half exists: the first DMA issues ~1.3us earlier
                    # and the four run on distinct queues.
                    for n in range(2):
                        cols = slice(512 * n, 512 * (n + 1))
                        nc.vector.tensor_scalar_add(osb[:, cols], ps[:, cols],
                                                    bias[m][:])
                        deng = [nc.sync, nc.scalar, nc.sync, nc.scalar][2 * m + n]
                        deng.dma_start(
                            out_d.ap()[128 * m : 128 * (m + 1), cols], osb[:, cols]
                        )

    nc.compile()
    return nc


def get_nc():
    if "nc" not in _cached:
        _cached["nc"] = build_nc()
    return _cached["nc"]


def make_in_maps(x, w_qkv, w_out, b_out):
    import ml_dtypes

    bf16 = ml_dtypes.bfloat16
    wqkvT = np.ascontiguousarray(w_qkv.T.astype(bf16))
    woutT = np.ascontiguousarray(w_out.T.astype(bf16))
    bias = np.ascontiguousarray(b_out.astype(np.float32).reshape(C, 1))
    in_maps = []
    for i in range(8):
        b, qh = i // 2, i % 2
        xbf = x[b].astype(bf16)
        # roll keys so this core's query half occupies columns 0:LQ; K and V
        # see the same permutation so attention output is unchanged.
        xrot = np.ascontiguousarray(np.roll(xbf, -qh * LQ, axis=1))
        in_maps.append({"x": xrot, "wqkvT": wqkvT, "woutT": woutT, "bias": bias})
    return in_maps


def assemble(results):
    out = np.empty((B, C, L), dtype=np.float32)
    for i in range(8):
        b, qh = i // 2, i % 2
        out[b][:, qh * LQ : (qh + 1) * LQ] = np.asarray(
            results[i]["out"], dtype=np.float32
        )
    return out


def kernel(x, w_qkv, w_out, b_out):
    x = np.asarray(x, dtype=np.float32)
    w_qkv = np.asarray(w_qkv, dtype=np.float32)
    w_out = np.asarray(w_out, dtype=np.float32)
    b_out = np.asarray(b_out, dtype=np.float32)
    assert x.shape == (B, C, L), x.shape
    nc = get_nc()
    in_maps = make_in_maps(x, w_qkv, w_out, b_out)
    res = run_bass_kernel_spmd(nc, in_maps, list(range(8)), trace=False)
    return assemble(res.results)

